# revision 39
# baseline (speedup 1.0000x reference)
"""Bahdanau attention Trainium2 kernel.

score(s, h_i) = v . tanh(W_s s + W_h h_i);  softmax over S;  context = w @ enc.

Strategy (per NeuronCore, data-parallel over batch, 8 batches/core):
  - enc (B,S,2H) f32 is cast to bf16 during the HBM->SBUF DMA (SWDGE cast).
  - natural bf16 tiles [s=128, 2H] feed the context GEMM (contract s);
    an SBUF->SBUF xbar transpose DMA produces encT tiles [h=128, s] that
    feed the projection GEMM (contract h) as the moving operand.
  - projection runs transposed: energyT[a=128part, s=512free] psum,
    W_h chunks (bf16) stationary.  dec_proj folds in as the per-partition
    ACT bias of the fused tanh.  scores = v . energyT via a K=a matmul.
  - softmax without max-subtraction (|scores| <~ 20, exp is safe in f32);
    unnormalized e accumulates the context in PSUM, 1/sum applied at the
    end on both outputs.
"""

import os
import sys
from contextlib import ExitStack

sys.path.insert(0, "/opt/trn_rl_repo")

import numpy as np

import concourse.bass as bass
import concourse.tile as tile
from concourse import mybir
from concourse.masks import make_identity

# ---- walrus workaround: tail drain accepts only 1 sync wait ----------------
from concourse.vector_clock import ScopedClock, VectorClock


def _patched_drain_and_barrier(self, tick_clock, wait_clock):
    gc = tick_clock.global_clock
    procs = [(i, gc[i]) for i in range(len(gc)) if gc[i] > 0]
    for p, t in procs:
        vc = VectorClock()
        vc.require_at_least(p, t)
        nop = self.nc.sync.nop(nofuse=True, hint="tail_wait_split")
        wait_clock.add_sem_waits(nop.ins, ScopedClock({None: vc}))
    self.nc.sync.drain()
    self.nc.all_engine_barrier()
    assert self.sems is not None
    popped = self.nc._tile_sem_poison_stack.pop()
    assert popped is self._sem_poison
    self.nc.clear_and_free_semaphores(list(self.sems.allocated().values()))
    self.nc.all_engine_barrier()


tile.TileContext._drain_and_barrier = _patched_drain_and_barrier


def _spill_excess_waits(nc):
    """This walrus build accepts at most 1 sync wait per instruction (2 for
    EventSemaphore).  Tile's wait assignment can attach several.  Move the
    excess onto same-engine NOPs inserted immediately before the
    instruction — NX sequencers process instructions in order, so the NOP
    stalls the engine exactly like an on-instruction wait would."""
    import bass_rust

    nop_id = [0]
    for fn in nc.m.functions:
        for blk in fn.blocks:
            new_insts = []
            changed = False
            for inst in blk.instructions:
                si = inst.sync_info
                cap = 2 if type(inst).__name__ == "InstEventSemaphore" else 1
                if si is not None and len(si.on_wait) > cap:
                    waits = list(si.on_wait)
                    keep, spill = waits[-cap:], waits[:-cap]
                    for w in spill:
                        nop = mybir.InstNoOp(
                            name=f"I-waitspill-{nop_id[0]}", ins=[], outs=[]
                        )
                        nop_id[0] += 1
                        nop.engine = inst.engine
                        nop.sync_info = bass_rust.SyncInfo(
                            on_wait=[w], on_update=[]
                        )
                        nc.register_instruction(nop, overwrite=True)
                        new_insts.append(nop)
                    inst.sync_info = bass_rust.SyncInfo(
                        on_wait=keep, on_update=list(si.on_update)
                    )
                    changed = True
                new_insts.append(inst)
            if changed:
                blk.instructions = new_insts
    return nc


# ---------------------------------------------------------------------------

N_CORES = 8
B, S, H, A = 64, 2048, 1024, 512
H2 = 2 * H
F32 = mybir.dt.float32
BF16 = mybir.dt.bfloat16
U8 = mybir.dt.uint8
AF = mybir.ActivationFunctionType
ALU = mybir.AluOpType


def build_bass(bloc, s_len, reps=1, coltile=True):
    """One-core program processing bloc batch rows of length s_len.

    reps>1 re-emits the whole compute loop (idempotent writes) so the
    per-iteration device time can be measured as a wall-clock slope."""
    P = 128
    SB = 512  # s-block
    n_blk = s_len // SB
    n_hc = H2 // P  # 16 h-chunks for the projection contraction
    n_ac = A // P  # 4 a-chunks
    n_hh = H2 // 512  # 4 context output slices
    n_wc = H // P  # 8 W_s chunks

    nc = bass.Bass("TRN2", target_bir_lowering=False, debug=False)
    enc = nc.dram_tensor("enc", [bloc, s_len, H2], F32, kind="ExternalInput").ap()
    dec = nc.dram_tensor("dec", [bloc, H], F32, kind="ExternalInput").ap()
    msk = nc.dram_tensor("msk", [bloc, s_len], U8, kind="ExternalInput").ap()
    w_s = nc.dram_tensor("w_s", [H, A], F32, kind="ExternalInput").ap()
    w_h = nc.dram_tensor("w_h", [H2, A], F32, kind="ExternalInput").ap()
    v_in = nc.dram_tensor("v_in", [1, A], F32, kind="ExternalInput").ap()
    cone = nc.dram_tensor("cone", [1, 1], F32, kind="ExternalInput").ap()
    csel = nc.dram_tensor("csel", [128, 1], F32, kind="ExternalInput").ap()
    ctx_o = nc.dram_tensor("ctx_o", [bloc, H2], F32, kind="ExternalOutput").ap()
    wgt_o = nc.dram_tensor("wgt_o", [bloc, s_len], F32, kind="ExternalOutput").ap()

    with tile.TileContext(nc) as tc, ExitStack() as ctx:
        consts = ctx.enter_context(tc.tile_pool(name="consts", bufs=1))

        # --- constants ---
        whsb = consts.tile([P, n_hc, A], BF16)
        nc.gpsimd.dma_start(whsb[:], w_h.rearrange("(c p) a -> p c a", p=P))
        vsb = consts.tile([P, n_ac], BF16)
        nc.gpsimd.dma_start(vsb[:], v_in.rearrange("o (c p) -> p (o c)", p=P))
        one_bf = consts.tile([1, 1], BF16)
        nc.gpsimd.dma_start(one_bf[:], cone[:])
        zcol = consts.tile([1, P], BF16)
        nc.gpsimd.memset(zcol[:], 0.0)
        zrow = consts.tile([1, 512], BF16)
        nc.gpsimd.memset(zrow[:], 0.0)
        # selector for summing the 4 col-group context partials (rows
        # 0/32/64/96): sel[p] = 1 at those partitions, else 0 (host input)
        sel_bf = consts.tile([P, 1], BF16)
        nc.gpsimd.dma_start(sel_bf[:], csel[:])

        # --- pools for the main loop ---
        nat_pool = ctx.enter_context(tc.tile_pool(name="nat", bufs=3))
        encT_pool = ctx.enter_context(tc.tile_pool(name="encT", bufs=2))

        def load_nat(b, blk):
            nat = nat_pool.tile([P, SB // P, H2], BF16, tag="nat")
            nc.gpsimd.dma_start(
                nat[:],
                enc[b, blk * SB : (blk + 1) * SB, :].rearrange(
                    "(q p) h -> p q h", p=P
                ),
            )
            return nat

        def make_encT(nat):
            # One xbar transpose per block: in_ = nat viewed [s=128,
            # q*2048 + c*128 + p_h], out[p, q*16+c, s] — which is exactly
            # the [p_h, q, c, s_local] layout, fully contiguous on both
            # sides (fast xbar path).  The energy matmul reads rhs strided
            # over (q, s_local).  (Issuing from the ACT HWDGE ring instead
            # was measured ~60us/iter slower — it serializes with tanh.)
            encT = encT_pool.tile([P, SB // P, n_hc, P], BF16, tag="encT")
            nc.sync.dma_start_transpose(
                encT.rearrange("p q c s -> p (q c) s"), nat[:]
            )
            return encT

        def load_block(b, blk):
            nat = load_nat(b, blk)
            return nat, make_encT(nat)

        # prefetch the first blocks before anything else queues on DMA
        prefetched = {}
        prefetched_nat = {}
        for b0, blk0 in [(0, 0), (0, 1)]:
            if bloc > b0 and n_blk > blk0:
                prefetched[(b0, blk0)] = load_block(b0, blk0)
        if n_blk > 2:
            prefetched_nat[(0, 2)] = load_nat(0, 2)

        # --- dec_projT[a, b] = sum_h W_s[h, a] * dec[b, h] ---
        dpt = consts.tile([P, n_ac, bloc], F32)
        with tc.tile_pool(name="setup", bufs=1) as setup, tc.tile_pool(
            name="setup_ps", bufs=1, space="PSUM"
        ) as setup_ps:
            wssb = setup.tile([P, n_wc, A], F32)
            nc.sync.dma_start(wssb[:], w_s.rearrange("(c p) a -> p c a", p=P))
            dec_nat = setup.tile([bloc, H], F32)
            nc.sync.dma_start(dec_nat[:], dec[:, :])
            ident = setup.tile([P, P], F32)
            make_identity(nc, ident[:])
            dect = setup.tile([P, n_wc, bloc], F32)
            for c in range(n_wc):
                tps = setup_ps.tile([P, bloc], F32, tag="tr_ps")
                nc.tensor.transpose(
                    tps[:], dec_nat[:, c * P : (c + 1) * P], ident[:bloc, :bloc]
                )
                nc.scalar.copy(dect[:, c, :], tps[:])
            for ca in range(n_ac):
                dps = setup_ps.tile([P, bloc], F32, tag="dp_ps")
                for c in range(n_wc):
                    nc.tensor.matmul(
                        dps[:],
                        wssb[:, c, ca * P : (ca + 1) * P],
                        dect[:, c, :],
                        start=(c == 0),
                        stop=(c == n_wc - 1),
                    )
                nc.scalar.copy(dpt[:, ca, :], dps[:])

        # --- more pools for the main loop ---
        et_pool = ctx.enter_context(tc.tile_pool(name="et", bufs=3))
        row_pool = ctx.enter_context(tc.tile_pool(name="row", bufs=2))
        out_pool = ctx.enter_context(tc.tile_pool(name="outp", bufs=2))
        energy_ps = ctx.enter_context(tc.tile_pool(name="energy_ps", bufs=2, space="PSUM"))
        small_ps = ctx.enter_context(tc.tile_pool(name="small_ps", bufs=2, space="PSUM"))
        ctx_ps = ctx.enter_context(tc.tile_pool(name="ctx_ps", bufs=1, space="PSUM"))

        # context psum lives across the whole kernel: col-group partials
        # land on rows 0/32/64/96; each batch zero-fills it via matmuls.
        cps = ctx_ps.tile([P, H2], F32, tag="cps")

        for b in [bb for _ in range(reps) for bb in range(bloc)]:
            m_u8 = row_pool.tile([1, s_len], U8, tag="m_u8")
            nc.sync.dma_start(m_u8[:], msk[b : b + 1, :])
            m_f32 = row_pool.tile([1, s_len], F32, tag="m_f32")
            nc.vector.tensor_copy(m_f32[:], m_u8[:])

            e_f32 = row_pool.tile([1, s_len], F32, tag="e_f32")
            e_bf = row_pool.tile([1, s_len], BF16, tag="e_bf")
            esum_p = row_pool.tile([1, n_blk], F32, tag="esum_p")

            # Zero the context banks (data + has_written bits, full M=128)
            # so every chain below can accumulate with start=False.  The
            # full-array matmul also hardware-serializes against the
            # col-tiled chains that follow.
            for hh in range(n_hh):
                nc.tensor.matmul(
                    cps[:, hh * 512 : (hh + 1) * 512],
                    zcol[:],
                    zrow[:],
                    start=True,
                    stop=False,
                    skip_group_check=True,
                )

            for blk in range(n_blk):
                pre = prefetched.pop((b, blk), None)
                if pre is not None:
                    nat, encT = pre
                else:
                    nat = prefetched_nat.pop((b, blk), None)
                    if nat is None:
                        nat = load_nat(b, blk)
                    encT = make_encT(nat)

                sps = small_ps.tile([1, SB], F32, tag="small")
                for ca in range(n_ac):
                    eps = energy_ps.tile([P, SB], F32, tag="eps")
                    for c in range(n_hc):
                        nc.tensor.matmul(
                            eps[:],
                            whsb[:, c, ca * P : (ca + 1) * P],
                            encT[:, :, c, :],
                            start=(c == 0),
                            stop=(c == n_hc - 1),
                        )
                    et = et_pool.tile([P, SB], BF16, tag="et")
                    nc.scalar.activation(
                        et[:], eps[:], AF.Tanh, bias=dpt[:, ca, b : b + 1]
                    )
                    nc.tensor.matmul(
                        sps[:],
                        vsb[:, ca : ca + 1],
                        et[:],
                        start=(ca == 0),
                        stop=(ca == n_ac - 1),
                    )

                # e = exp(scores) (no max-sub: |scores| small), then mask and
                # accumulate the block sum in one DVE op.
                esl = slice(blk * SB, (blk + 1) * SB)
                nc.scalar.activation(e_f32[0:1, esl], sps[:], AF.Exp)
                nc.vector.tensor_mul(e_f32[0:1, esl], e_f32[0:1, esl], m_f32[0:1, esl])
                nc.vector.reduce_sum(
                    esum_p[0:1, blk : blk + 1],
                    e_f32[0:1, esl],
                    axis=mybir.AxisListType.X,
                )
                nc.vector.tensor_copy(e_bf[0:1, esl], e_f32[0:1, esl])

                # transpose e into [s=128, 1] columns via K=1 matmuls
                etr = small_ps.tile([P, SB // P], F32, tag="small")
                for q in range(SB // P):
                    nc.tensor.matmul(
                        etr[:, q : q + 1],
                        e_bf[0:1, blk * SB + q * P : blk * SB + (q + 1) * P],
                        one_bf[:],
                        start=True,
                        stop=True,
                    )
                etr_bf = et_pool.tile([P, SB // P], BF16, tag="etr_bf")
                nc.scalar.copy(etr_bf[:], etr[:])

                # context accumulation: ctx[h] += sum_s e[s] * enc[s, h].
                # The four q-subtiles run in separate PE column groups
                # (tile_position), so their N=512 streams overlap on HW;
                # partials land on psum rows 0/32/64/96 and are summed at
                # the end of the batch.
                for q in range(SB // P):
                    for hh in range(n_hh):
                        row = 32 * q if coltile else 0
                        nc.tensor.matmul(
                            cps[row : row + 1, hh * 512 : (hh + 1) * 512],
                            etr_bf[:, q : q + 1],
                            nat[:, q, hh * 512 : (hh + 1) * 512],
                            start=False,
                            stop=(blk == n_blk - 1),
                            tile_position=(0, row) if coltile else None,
                            skip_group_check=True,
                        )

            esum = row_pool.tile([1, 1], F32, tag="esum")
            nc.vector.reduce_sum(esum[:], esum_p[:], axis=mybir.AxisListType.X)
            inv = row_pool.tile([1, 1], F32, tag="inv")
            nc.vector.reciprocal(inv[:], esum[:])

            # sum the 4 col-group partial rows: c4 = cps (bf16 copy), then
            # ctx[h] = sel . c4[:, h] via a K=128 matmul
            ctx_sb = out_pool.tile([1, H2], F32, tag="ctx_sb")
            if coltile:
                c4 = out_pool.tile([P, H2], BF16, tag="c4")
                nc.scalar.copy(c4[:], cps[:])
                for hh in range(n_hh):
                    c2 = small_ps.tile([1, 512], F32, tag="small")
                    nc.tensor.matmul(
                        c2[:],
                        sel_bf[:],
                        c4[:, hh * 512 : (hh + 1) * 512],
                        start=True,
                        stop=True,
                    )
                    nc.scalar.activation(
                        ctx_sb[0:1, hh * 512 : (hh + 1) * 512],
                        c2[:],
                        AF.Copy,
                        scale=inv[:],
                    )
            else:
                for hh in range(n_hh):
                    nc.scalar.activation(
                        ctx_sb[0:1, hh * 512 : (hh + 1) * 512],
                        cps[0:1, hh * 512 : (hh + 1) * 512],
                        AF.Copy,
                        scale=inv[:],
                    )
            wgt_sb = out_pool.tile([1, s_len], F32, tag="wgt_sb")
            nc.scalar.activation(wgt_sb[:], e_f32[:], AF.Copy, scale=inv[:])

            nc.sync.dma_start(ctx_o[b : b + 1, :], ctx_sb[:])
            nc.sync.dma_start(wgt_o[b : b + 1, :], wgt_sb[:])

    return _spill_excess_waits(nc)


class _Runner:
    """Compile once, execute many times with device-resident inputs."""

    def __init__(self, bloc, s_len, n_cores=N_CORES):
        import jax
        from jax.experimental.shard_map import shard_map
        from jax.sharding import Mesh, PartitionSpec

        from concourse import bass2jax

        bass2jax.install_neuronx_cc_hook()
        self.n_cores = n_cores
        self.bloc = bloc
        nc = build_bass(bloc, s_len)
        in_names, out_names, out_avals = [], [], []
        for alloc in nc.m.functions[0].allocations:
            if not isinstance(alloc, mybir.MemoryLocationSet):
                continue
            name = alloc.memorylocations[0].name
            if alloc.kind == "ExternalInput":
                in_names.append(name)
            elif alloc.kind == "ExternalOutput":
                out_names.append(name)
                out_avals.append(
                    jax.core.ShapedArray(
                        tuple(alloc.tensor_shape), mybir.dt.np(alloc.dtype)
                    )
                )
        partition_name = (
            nc.partition_id_tensor.name if nc.partition_id_tensor else None
        )
        if partition_name is not None:
            in_names = [n for n in in_names if n != partition_name]
        self.in_names = in_names
        self.out_names = out_names
        self.out_avals = out_avals
        n_params = len(in_names)
        n_outs = len(out_names)
        all_in_names = tuple(in_names) + tuple(out_names)
        if partition_name is not None:
            all_in_names = all_in_names + (partition_name,)

        def _body(*args):
            operands = list(args)
            if partition_name is not None:
                operands.append(bass2jax.partition_id_tensor())
            outs = bass2jax._bass_exec_p.bind(
                *operands,
                out_avals=tuple(out_avals),
                in_names=all_in_names,
                out_names=tuple(out_names),
                lowering_input_output_aliases=(),
                sim_require_finite=True,
                sim_require_nnan=True,
                nc=nc,
            )
            return tuple(outs)

        devices = jax.devices()[:n_cores]
        self.mesh = Mesh(np.asarray(devices), ("core",))
        in_specs = (PartitionSpec("core"),) * (n_params + n_outs)
        out_specs = (PartitionSpec("core"),) * n_outs
        self.sharded = jax.jit(
            shard_map(
                _body,
                mesh=self.mesh,
                in_specs=in_specs,
                out_specs=out_specs,
                check_rep=False,
            ),
            donate_argnums=tuple(range(n_params, n_params + n_outs)),
            keep_unused=True,
        )
        self._jax = jax

    def put_inputs(self, per_core_maps):
        """per_core_maps: list of dicts name->np array (per-core shapes).
        Returns device arrays (concatenated on axis 0)."""
        import jax
        from jax.sharding import NamedSharding, PartitionSpec

        sh = NamedSharding(self.mesh, PartitionSpec("core"))
        arrs = []
        for name in self.in_names:
            cat = np.concatenate(
                [np.asarray(m[name]) for m in per_core_maps], axis=0
            )
            arrs.append(jax.device_put(cat, sh))
        jax.block_until_ready(arrs)
        return arrs

    def _zero_outs(self):
        return [
            np.zeros((self.n_cores * a.shape[0], *a.shape[1:]), a.dtype)
            for a in self.out_avals
        ]

    def run(self, dev_inputs):
        outs = self.sharded(*dev_inputs, *self._zero_outs())
        self._jax.block_until_ready(outs)
        return outs

    def run_np(self, dev_inputs):
        outs = self.run(dev_inputs)
        return {n: np.asarray(o) for n, o in zip(self.out_names, outs)}


_RUNNER_CACHE = {}


def _get_runner(bloc, s_len, n_cores=N_CORES):
    key = (bloc, s_len, n_cores)
    if key not in _RUNNER_CACHE:
        _RUNNER_CACHE[key] = _Runner(bloc, s_len, n_cores)
    return _RUNNER_CACHE[key]


def make_in_maps(decoder_hidden, encoder_outputs, mask, W_s, W_h, v, n_cores=N_CORES):
    b_full = mask.shape[0]
    bloc = b_full // n_cores
    enc_np = np.ascontiguousarray(encoder_outputs, dtype=np.float32)
    dec_np = np.ascontiguousarray(decoder_hidden, dtype=np.float32)
    msk_np = np.ascontiguousarray(mask).view(np.uint8)
    ws_np = np.ascontiguousarray(W_s, dtype=np.float32)
    wh_np = np.ascontiguousarray(W_h, dtype=np.float32)
    v_np = np.ascontiguousarray(v, dtype=np.float32).reshape(1, -1)
    one_np = np.ones((1, 1), np.float32)
    sel_np = np.zeros((128, 1), np.float32)
    sel_np[::32] = 1.0
    in_maps = []
    for i in range(n_cores):
        sl = slice(i * bloc, (i + 1) * bloc)
        in_maps.append(
            {
                "enc": enc_np[sl],
                "dec": dec_np[sl],
                "msk": msk_np[sl],
                "w_s": ws_np,
                "w_h": wh_np,
                "v_in": v_np,
                "cone": one_np,
                "csel": sel_np,
            }
        )
    return in_maps


def run_sharded(decoder_hidden, encoder_outputs, mask, W_s, W_h, v, n_cores=N_CORES):
    b_full, s_len = mask.shape
    bloc = b_full // n_cores
    runner = _get_runner(bloc, s_len, n_cores)
    in_maps = make_in_maps(
        decoder_hidden, encoder_outputs, mask, W_s, W_h, v, n_cores
    )
    dev_in = runner.put_inputs(in_maps)
    outs = runner.run_np(dev_in)
    ctx = outs["ctx_o"].reshape(b_full, H2)
    wgt = outs["wgt_o"].reshape(b_full, s_len)
    return ctx, wgt


def kernel(decoder_hidden, encoder_outputs, mask, W_s, W_h, v):
    decoder_hidden = np.asarray(decoder_hidden)
    encoder_outputs = np.asarray(encoder_outputs)
    mask = np.asarray(mask)
    W_s = np.asarray(W_s)
    W_h = np.asarray(W_h)
    v = np.asarray(v)
    ctx, wgt = run_sharded(decoder_hidden, encoder_outputs, mask, W_s, W_h, v)
    return ctx, wgt
